# revision 12
# baseline (speedup 1.0000x reference)
"""Squeeze-and-Excitation attention module on 8 Trainium2 NeuronCores.

Reference computation (per image b):
    y[c]  = mean(x[b, c, :, :])                      # global average pool
    z     = relu(w1 @ y + b1)                        # FC 512 -> 32
    s     = sigmoid(w2 @ z + b2)                     # FC 32 -> 512
    out[b, c, :, :] = x[b, c, :, :] * s[c]

Sharding: data-parallel over batch. 32 images / 8 cores = 4 images per
core; the tiny FC weights are replicated. Each core streams its images
through SBUF once (load -> pool -> FCs -> in-place scale -> store), so
HBM traffic is the 2x33.5 MB minimum per core.

Layouts (prepared host-side, all free for 68 KB of weights):
    x      [4, 512, 4096]  per-core shard, spatial flattened
    w1t    [128, 4, 32]    w1t[p, k, r] = w1[r, 128k + p]
    b1     [32, 1]
    w2t    [32, 4, 128]    w2t[r, k, p] = w2[128k + p, r]
    b2c    [128, 4]        b2c[p, k]   = b2[128k + p]

Per image, channels are split into 4 chunks of 128 (one SBUF tile
[128, 4096] = 2 MB each). Loads go out on the Sync HWDGE queue and
stores on the GpSimd SWDGE queue so a store waiting on compute never
head-of-line-blocks the next image's loads. Pooling reduces are split
between DVE (tensor_reduce) and ACT (in-place Copy + accum_out); the
scale multiplies are split the other way. Emission is software-
pipelined one image deep: image b's scale/store instructions are
emitted after image b+1's load/reduce so neither engine stream stalls
on the other image's dependencies.
"""

import numpy as np

B = 32
C = 512
HW = 64 * 64
N_CORES = 8
B_LOC = B // N_CORES
KC = C // 128  # channel chunks of 128

_NC_CACHE = {}

# Set by test harness to capture a profile; harmless default for grading.
TRACE = False
LAST_RESULT = None


def _build_nc():
    from contextlib import ExitStack

    import concourse.tile as tile
    from concourse import bacc, mybir

    f32 = mybir.dt.float32
    AF = mybir.ActivationFunctionType
    nc = bacc.Bacc("TRN2", target_bir_lowering=False, debug=False)

    # x is indexed [image, chunk-pair, channel-in-chunk, half, spatial]:
    # channel chunks of 128 are grouped in pairs so one 4 MB DMA moves
    # two chunks ([128, 2, HW] tile).
    x = nc.dram_tensor("x", [B_LOC, KC // 2, 2, 128, HW], f32, kind="ExternalInput")
    w1t = nc.dram_tensor("w1t", [128, KC, 32], f32, kind="ExternalInput")
    b1 = nc.dram_tensor("b1", [32, 1], f32, kind="ExternalInput")
    w2t = nc.dram_tensor("w2t", [32, KC, 128], f32, kind="ExternalInput")
    b2c = nc.dram_tensor("b2c", [128, KC], f32, kind="ExternalInput")
    out = nc.dram_tensor(
        "out", [B_LOC, KC // 2, 2, 128, HW], f32, kind="ExternalOutput"
    )

    with ExitStack() as ctx:
        tc = ctx.enter_context(tile.TileContext(nc))
        singles = ctx.enter_context(tc.tile_pool(name="singles", bufs=1))
        xpool = ctx.enter_context(tc.tile_pool(name="xpool", bufs=5))
        small = ctx.enter_context(tc.tile_pool(name="small", bufs=2))
        psum = ctx.enter_context(tc.tile_pool(name="psum", bufs=2, space="PSUM"))

        w1t_sb = singles.tile([128, KC, 32], f32)
        b1_sb = singles.tile([32, 1], f32)
        w2t_sb = singles.tile([32, KC, 128], f32)
        b2_sb = singles.tile([128, KC], f32)

        NP = KC // 2  # chunk pairs per image
        for b in range(B_LOC):
            xts = []
            for p in range(NP):
                xt = xpool.tile([128, 2, HW], f32, tag="x")
                nc.sync.dma_start(out=xt, in_=x[b, p].rearrange("h p j -> p h j"))
                xts.append(xt)

            if b == 0:
                # Weight loads ride the otherwise-idle SWDGE queue so
                # they never delay image loads on the Sync ring; they
                # are only needed once pooling finishes.
                nc.gpsimd.dma_start(out=w1t_sb, in_=w1t[:])
                nc.gpsimd.dma_start(out=b1_sb, in_=b1[:])
                nc.gpsimd.dma_start(out=w2t_sb, in_=w2t[:])
                nc.gpsimd.dma_start(out=b2_sb, in_=b2c[:])

            zp = psum.tile([32, 1], f32, tag="z")
            for p in range(NP):
                sums = small.tile([128, 2], f32, tag=f"sum{p}")
                nc.vector.tensor_reduce(
                    out=sums,
                    in_=xts[p],
                    axis=mybir.AxisListType.X,
                    op=mybir.AluOpType.add,
                )
                for h in range(2):
                    k = 2 * p + h
                    nc.tensor.matmul(
                        zp,
                        lhsT=w1t_sb[:, k, :],
                        rhs=sums[:, h : h + 1],
                        start=(k == 0),
                        stop=(k == KC - 1),
                    )

            z = small.tile([32, 1], f32, tag="z_sb")
            nc.scalar.activation(z, zp, AF.Relu, bias=b1_sb, scale=1.0 / HW)

            sp = psum.tile([128, KC], f32, tag="s")
            s_tiles = []
            for k in range(KC):
                nc.tensor.matmul(
                    sp[:, k : k + 1],
                    lhsT=w2t_sb[:, k, :],
                    rhs=z,
                    start=True,
                    stop=True,
                )
            for k in range(KC):
                s = small.tile([128, 1], f32, tag=f"s{k}")
                nc.scalar.activation(
                    s, sp[:, k : k + 1], AF.Sigmoid, bias=b2_sb[:, k : k + 1]
                )
                s_tiles.append(s)

            # Scale in place and store. ACT handles the multiplies (DVE
            # stays dedicated to pooling so it never blocks on this
            # image's sigmoid); the last image splits them across both
            # engines to shorten the drain tail. Stores ride the SWDGE
            # queue so a store waiting on compute never head-of-line-
            # blocks the Sync load ring.
            last = b == B_LOC - 1
            for p in range(NP):
                for h in range(2):
                    k = 2 * p + h
                    if last and p == 1:
                        nc.vector.tensor_scalar_mul(
                            xts[p][:, h, :], xts[p][:, h, :], s_tiles[k]
                        )
                    else:
                        nc.scalar.mul(
                            xts[p][:, h, :], xts[p][:, h, :], s_tiles[k]
                        )
                nc.gpsimd.dma_start(
                    out=out[b, p].rearrange("h p j -> p h j"), in_=xts[p]
                )

    nc.compile()
    return nc


def _get_nc():
    if "nc" not in _NC_CACHE:
        _NC_CACHE["nc"] = _build_nc()
    return _NC_CACHE["nc"]


def kernel(x, w1, b1, w2, b2):
    global LAST_RESULT
    from concourse.bass_utils import run_bass_kernel_spmd

    xf = np.ascontiguousarray(
        x.reshape(B, KC // 2, 2, 128, HW), dtype=np.float32
    )
    w1t = np.ascontiguousarray(w1.reshape(32, KC, 128).transpose(2, 1, 0))
    b1c = np.ascontiguousarray(b1.reshape(32, 1))
    w2t = np.ascontiguousarray(w2.reshape(KC, 128, 32).transpose(2, 0, 1))
    b2c = np.ascontiguousarray(b2.reshape(KC, 128).T)

    in_maps = [
        {
            "x": np.ascontiguousarray(xf[i * B_LOC : (i + 1) * B_LOC]),
            "w1t": w1t,
            "b1": b1c,
            "w2t": w2t,
            "b2c": b2c,
        }
        for i in range(N_CORES)
    ]

    nc = _get_nc()
    res = run_bass_kernel_spmd(
        nc, in_maps, core_ids=list(range(N_CORES)), trace=TRACE
    )
    LAST_RESULT = res
    out = np.concatenate([r["out"] for r in res.results], axis=0)
    return out.reshape(B, C, 64, 64)


# revision 14
# speedup vs baseline: 1.0459x; 1.0459x over previous
"""Squeeze-and-Excitation attention module on 8 Trainium2 NeuronCores.

Reference computation (per image b):
    y[c]  = mean(x[b, c, :, :])                      # global average pool
    z     = relu(w1 @ y + b1)                        # FC 512 -> 32
    s     = sigmoid(w2 @ z + b2)                     # FC 32 -> 512
    out[b, c, :, :] = x[b, c, :, :] * s[c]

Sharding: data-parallel over batch. 32 images / 8 cores = 4 images per
core; the tiny FC weights are replicated. Each core streams its images
through SBUF once (load -> pool -> FCs -> in-place scale -> store), so
HBM traffic per core is the 2 x 33.5 MB minimum, which at the ~430 GB/s
SBUF-port fabric ceiling bounds the kernel at ~156 us.

Layouts (prepared host-side, all free for 68 KB of weights):
    x      [4, 4, 128, 4096]  per-core shard: image, channel-chunk,
                              channel-in-chunk, flattened spatial
    w1t    [128, 4, 32]       w1t[p, k, r] = w1[r, 128k + p]
    b1     [32, 1]
    w2t    [32, 4, 128]       w2t[r, k, p] = w2[128k + p, r]
    b2c    [128, 4]           b2c[p, k]   = b2[128k + p]

Per image, channels split into 4 chunks of 128 (one SBUF tile
[128, 4096] = 2 MB each; 10-slot pool = 2.5 images in flight). ALL
image DMA rides the single Sync HWDGE ring in the FIFO order
L(0) L(1) S(0) L(2) S(1) L(3) S(2) S(3): by the time the sequencer
reaches the S(b) triggers, image b's scale factors are long since
computed, so the ring never stalls and runs at the fabric ceiling with
no dual-queue arbitration variance. Pooling runs on DVE only (its
stream is never blocked by scale work); scale multiplies + FC
activations run on ACT, emitted for image b-1 BEFORE image b's
relu/sigmoid so they start as soon as the sigmoid of their own image
is done. The last image splits its multiplies across ACT and DVE to
shorten the drain tail. Weight loads ride the otherwise-idle SWDGE
queue.
"""

import numpy as np

B = 32
C = 512
HW = 64 * 64
N_CORES = 8
B_LOC = B // N_CORES
KC = C // 128  # channel chunks of 128

_NC_CACHE = {}

# Set by test harness to capture a profile; harmless default for grading.
TRACE = False
LAST_RESULT = None


def _build_nc():
    from contextlib import ExitStack

    import concourse.tile as tile
    from concourse import bacc, mybir

    f32 = mybir.dt.float32
    AF = mybir.ActivationFunctionType
    nc = bacc.Bacc("TRN2", target_bir_lowering=False, debug=False)

    x = nc.dram_tensor("x", [B_LOC, KC, 128, HW], f32, kind="ExternalInput")
    w1t = nc.dram_tensor("w1t", [128, KC, 32], f32, kind="ExternalInput")
    b1 = nc.dram_tensor("b1", [32, 1], f32, kind="ExternalInput")
    w2t = nc.dram_tensor("w2t", [32, KC, 128], f32, kind="ExternalInput")
    b2c = nc.dram_tensor("b2c", [128, KC], f32, kind="ExternalInput")
    out = nc.dram_tensor("out", [B_LOC, KC, 128, HW], f32, kind="ExternalOutput")

    with ExitStack() as ctx:
        tc = ctx.enter_context(tile.TileContext(nc))
        singles = ctx.enter_context(tc.tile_pool(name="singles", bufs=1))
        xpool = ctx.enter_context(tc.tile_pool(name="xpool", bufs=10))
        small = ctx.enter_context(tc.tile_pool(name="small", bufs=2))
        psum = ctx.enter_context(tc.tile_pool(name="psum", bufs=2, space="PSUM"))

        w1t_sb = singles.tile([128, KC, 32], f32)
        b1_sb = singles.tile([32, 1], f32)
        w2t_sb = singles.tile([32, KC, 128], f32)
        b2_sb = singles.tile([128, KC], f32)

        prev = None  # (b, xts, s_tiles) of the image awaiting scale+store

        def emit_scale_and_store(state, split_engines):
            pb, xts, s_tiles = state
            for k in range(KC):
                if split_engines and k >= 2:
                    nc.vector.tensor_scalar_mul(xts[k], xts[k], s_tiles[k])
                else:
                    nc.scalar.mul(xts[k], xts[k], s_tiles[k])
                nc.sync.dma_start(out=out[pb, k], in_=xts[k])

        for b in range(B_LOC):
            xts = []
            for k in range(KC):
                xt = xpool.tile([128, HW], f32, tag="x")
                nc.sync.dma_start(out=xt, in_=x[b, k])
                xts.append(xt)

            if b == 0:
                nc.gpsimd.dma_start(out=w1t_sb, in_=w1t[:])
                nc.gpsimd.dma_start(out=b1_sb, in_=b1[:])
                nc.gpsimd.dma_start(out=w2t_sb, in_=w2t[:])
                nc.gpsimd.dma_start(out=b2_sb, in_=b2c[:])

            zp = psum.tile([32, 1], f32, tag="z")
            for k in range(KC):
                sums = small.tile([128, 1], f32, tag=f"sum{k}")
                nc.vector.tensor_reduce(
                    out=sums,
                    in_=xts[k],
                    axis=mybir.AxisListType.X,
                    op=mybir.AluOpType.add,
                )
                nc.tensor.matmul(
                    zp,
                    lhsT=w1t_sb[:, k, :],
                    rhs=sums,
                    start=(k == 0),
                    stop=(k == KC - 1),
                )

            # Previous image's scale+store: on ACT before this image's
            # relu/sigmoid (so it isn't gated on this image's pooling),
            # and on the Sync ring after this image's load triggers
            # (so the ring alternates L(b) then S(b-1)).
            if prev is not None:
                emit_scale_and_store(prev, split_engines=False)

            z = small.tile([32, 1], f32, tag="z_sb")
            nc.scalar.activation(z, zp, AF.Relu, bias=b1_sb, scale=1.0 / HW)

            sp = psum.tile([128, KC], f32, tag="s")
            s_tiles = []
            for k in range(KC):
                nc.tensor.matmul(
                    sp[:, k : k + 1],
                    lhsT=w2t_sb[:, k, :],
                    rhs=z,
                    start=True,
                    stop=True,
                )
            for k in range(KC):
                s = small.tile([128, 1], f32, tag=f"s{k}")
                nc.scalar.activation(
                    s, sp[:, k : k + 1], AF.Sigmoid, bias=b2_sb[:, k : k + 1]
                )
                s_tiles.append(s)

            prev = (b, xts, s_tiles)

        emit_scale_and_store(prev, split_engines=True)

    nc.compile()
    return nc


def _get_nc():
    if "nc" not in _NC_CACHE:
        _NC_CACHE["nc"] = _build_nc()
    return _NC_CACHE["nc"]


def kernel(x, w1, b1, w2, b2):
    global LAST_RESULT
    from concourse.bass_utils import run_bass_kernel_spmd

    xf = np.ascontiguousarray(x.reshape(B, KC, 128, HW), dtype=np.float32)
    w1t = np.ascontiguousarray(w1.reshape(32, KC, 128).transpose(2, 1, 0))
    b1c = np.ascontiguousarray(b1.reshape(32, 1))
    w2t = np.ascontiguousarray(w2.reshape(KC, 128, 32).transpose(2, 0, 1))
    b2c = np.ascontiguousarray(b2.reshape(KC, 128).T)

    in_maps = [
        {
            "x": np.ascontiguousarray(xf[i * B_LOC : (i + 1) * B_LOC]),
            "w1t": w1t,
            "b1": b1c,
            "w2t": w2t,
            "b2c": b2c,
        }
        for i in range(N_CORES)
    ]

    nc = _get_nc()
    res = run_bass_kernel_spmd(
        nc, in_maps, core_ids=list(range(N_CORES)), trace=TRACE
    )
    LAST_RESULT = res
    out = np.concatenate([r["out"] for r in res.results], axis=0)
    return out.reshape(B, C, 64, 64)
